# revision 18
# baseline (speedup 1.0000x reference)
"""Masked multi-head attention on 8 NeuronCores (faithful torch raw-view semantics).

The reference reshapes (bs, sql, nh*edim) -> (bs, nh, sql, edim) as a RAW VIEW:
head h's length-1024 pseudo-sequence comes from x rows 128h..128h+127, each row
contributing 8 pseudo-positions s' = 8r + cb (cb = 256-col block of the
projection). Work splits into 32 independent (batch, head) pairs -> 4 per core.

v3 design (vs the fp32r/bf16 baseline):
- Projections run as fp8e4m3 DoubleRow matmuls with a 3-product residual
  scheme (x8*w8 + rx8*(w8/8) + (x8/8)*rw8, residuals pre-scaled by 8 on the
  host): 256-deep contraction in one PE pass at 0.5 cycles/row, 1.5N rows
  per tile vs 2N for bf16. All six operand tensors are host-prepped fp8 -
  zero on-device conversion cost.
- Q is NOT pre-scaled; the 1/16 softmax scale is folded into the exp
  activation's scale argument (fp8 weights would underflow at sigma=1/256).
- Scores/PV/denominator in bf16 (removes the fp32r >=256-moving-column
  padding). Causal triangle is ADDED ON THE PE (idn x tri accumulate into
  the score PSUM) instead of DVE tensor_adds.
- V^T per-head key-block transposes run on the DMA XBAR (dma_start_transpose,
  14ns/16x128-tile) instead of PE transposes + DVE copybacks.
- PSUM: score tiles [128,512] bufs=2, po bufs=4 (so consecutive qj rounds
  overlap), se, yp.
- Output projection per head (full r), emitted one head late into the PE
  stream so it runs while DVE normalizes the next head's first rounds.
"""

import sys

sys.path.insert(0, "/opt/trn_rl_repo")

import ml_dtypes
import numpy as np

from concourse import bacc, mybir
from concourse.tile import TileContext
from concourse.bass_utils import run_bass_kernel_spmd

EDIM = 256
BS = 4
SQL = 1024
HPC = 4           # heads per core
NCORES = 8
FDT = mybir.dt.float32
BDT = mybir.dt.bfloat16
QDT = mybir.dt.float8e4
NEG = -1.0e30
RS = 8.0          # fp8 residual pre-scale
DR = mybir.MatmulPerfMode.DoubleRow

_cache = {}


def _build():
    nc = bacc.Bacc(dynamic_dma_scratch_size=512)

    # fp8 moving operands (x^T), layout [p, i*512+n] = x_slice.T[i*128+p, n]
    # packed [x8 | fp8(rx) | x8/8]
    xpk = nc.declare_dram_parameter("xpk", [128, 3072], QDT, isOutput=False)
    # fp8 stationary weights, col = ti*256 + i*128 + m; ti = s*16 + cb*2 + c
    # sections s in [V, Q, K]; wr8 = (w - w8)*8
    w8p = nc.declare_dram_parameter("w8p", [128, 12288], QDT, isOutput=False)
    wr8p = nc.declare_dram_parameter("wr8p", [128, 12288], QDT, isOutput=False)
    # bf16 constants: idn | tri | ones
    cst = nc.declare_dram_parameter("cst", [128, 384], BDT, isOutput=False)
    bqkv = nc.declare_dram_parameter("bqkv", [128, 56], FDT, isOutput=False)
    wot = nc.declare_dram_parameter("wot", [128, 4096], BDT, isOutput=False)
    y = nc.declare_dram_parameter("y", [512, 256], FDT, isOutput=True)

    EXP = mybir.ActivationFunctionType.Exp

    with TileContext(nc) as tc:
        with (
            tc.tile_pool(name="const", bufs=1) as cpool,
            tc.tile_pool(name="qk", bufs=1) as qkpool,
            tc.tile_pool(name="ohp", bufs=2) as ohpool,
            tc.tile_pool(name="vkp", bufs=4) as vkpool,
            tc.tile_pool(name="ptp", bufs=4) as ptpool,
            tc.tile_pool(name="work", bufs=2) as wpool,
        ):
            def mm(out, lhsT, rhs, **kw):
                nc.tensor.matmul(out, lhsT, rhs, **kw)

            def tile_of(pool, name, shape, dt=FDT, tag=None):
                return pool.tile(shape, dt, tag=tag or name, name=name)

            # ---- input DMA: split across the two HWDGE queues (SP, Act) in
            # need order; W tensors per-section so arrival chases the PE.
            xpk_sb = tile_of(cpool, "xpk", [128, 3072], dt=QDT)
            w8_sb = tile_of(cpool, "w8", [128, 12288], dt=QDT)
            wr8_sb = tile_of(cpool, "wr8", [128, 12288], dt=QDT)
            cst_sb = tile_of(cpool, "cst", [128, 384], dt=BDT)
            bqkv_sb = tile_of(cpool, "bqkv", [128, 56])
            wot_sb = tile_of(cpool, "wot", [128, 4096], dt=BDT)

            # SP first (the Act queue's SEQ is blocked ~1.3us by the act
            # table load): x pack + first V chunks, then Q; Act: rest of V,
            # K, consts. Transfers serialize on the single DMA_ENGINES
            # device in roughly this interleaved order.
            nc.sync.dma_start(out=xpk_sb[:, :], in_=xpk[:, :])
            nc.scalar.dma_start(out=bqkv_sb[:, :], in_=bqkv[:, :])
            nc.scalar.dma_start(out=cst_sb[:, :], in_=cst[:, :])
            for lo in range(0, 12288, 2048):
                nc.sync.dma_start(out=w8_sb[:, lo:lo + 2048],
                                  in_=w8p[:, lo:lo + 2048])
                nc.sync.dma_start(out=wr8_sb[:, lo:lo + 2048],
                                  in_=wr8p[:, lo:lo + 2048])
            nc.sync.dma_start(out=wot_sb[:, :], in_=wot[:, :])

            idn_sb = cst_sb[:, 0:128]
            tri_sb = cst_sb[:, 128:256]
            onc_sb = cst_sb[:, 256:384]

            # fp8 moving views [p, i, n] for DoubleRow
            x8_r = xpk_sb[:, 0:1024].rearrange("p (i n) -> p i n", i=2)
            xr8_r = xpk_sb[:, 1024:2048].rearrange("p (i n) -> p i n", i=2)
            x8s_r = xpk_sb[:, 2048:3072].rearrange("p (i n) -> p i n", i=2)

            # natural-order projections: col = h*1024 + 8r + cb
            qt = [qkpool.tile([128, 4096], BDT, tag=f"qt{c}", name=f"qt{c}")
                  for c in range(2)]
            kt = [qkpool.tile([128, 4096], BDT, tag=f"kt{c}", name=f"kt{c}")
                  for c in range(2)]
            vt = [qkpool.tile([128, 4096], BDT, tag=f"vt{c}", name=f"vt{c}")
                  for c in range(2)]

            # ---- P1: V, Q, K projections as fp8 DoubleRow quads ----
            # quad (s, j, c) = 4 units (cb = 4j..4j+4, fixed c) in one
            # 4-bank PSUM tile; DVE drains a quad in ONE merged
            # tensor_tensor (broadcast bias), Act drains per-unit.
            vk = {}

            def emit_transposes(hl):
                vkt = vkpool.tile([128, 2048], BDT, tag="vk", name=f"vk{hl}")
                for c in range(2):
                    nc.sync.dma_start_transpose(
                        out=vkt[:, c * 1024:(c + 1) * 1024].rearrange(
                            "p (b d) -> p b d", b=8),
                        in_=vt[c][:, hl * 1024:(hl + 1) * 1024],
                    )
                vk[hl] = vkt

            quads = [(s, j, c) for s in range(3) for j in range(2)
                     for c in range(2)]
            ACT_QUADS = {1, 4, 6, 8, 10}   # drained as 4 Act singles

            with tc.tile_pool(name="ps_pj", bufs=2, space="PSUM") as ps_pj:
                # PE warmup: throwaway matmuls on a memset tile ramp the PE
                # to full pstate while the first weight DMAs are in flight.
                wup = wpool.tile([32, 640], BDT, tag="wup", name="wup",
                                 bufs=1)
                nc.vector.memset(wup[:, 0:320], 0.0)
                nc.gpsimd.memset(wup[:, 320:640], 0.0)
                wps = ps_pj.tile([128, 2048], FDT, tag="ps", name="warm")
                for _ in range(8):
                    mm(wps[:, 0:512], wup[:, 512:640], wup[:, 0:512])

                for qi, (s, j, c) in enumerate(quads):
                    ps = ps_pj.tile([128, 2048], FDT, tag="ps", name="proj")
                    for t in range(4):
                        cb = 4 * j + t
                        ti = s * 16 + cb * 2 + c
                        wof = ti * 256
                        reg = ps[:, t * 512:(t + 1) * 512]
                        w_a = w8_sb[:, wof:wof + 256].rearrange(
                            "p (i m) -> p i m", i=2)
                        w_c = wr8_sb[:, wof:wof + 256].rearrange(
                            "p (i m) -> p i m", i=2)
                        mm(reg, w_a, x8_r, start=True, stop=False,
                           perf_mode=DR)
                        mm(reg, w_a, xr8_r, start=False, stop=False,
                           perf_mode=DR)
                        mm(reg, w_c, x8s_r, start=False, stop=True,
                           perf_mode=DR)
                    dst = (vt, qt, kt)[s]
                    ti0 = s * 16 + 8 * j + c
                    if qi in ACT_QUADS:
                        for t in range(4):
                            cb = 4 * j + t
                            ti = s * 16 + cb * 2 + c
                            out_ap = dst[c].rearrange(
                                "p (h r e) -> p h r e", h=4, r=128
                            )[:, :, :, cb]
                            in_ap = ps[:, t * 512:(t + 1) * 512].rearrange(
                                "p (h r) -> p h r", h=4)
                            nc.scalar.add(out_ap, in_ap,
                                          add=bqkv_sb[:, ti:ti + 1])
                    else:
                        out_ap = dst[c].rearrange(
                            "p (h r e) -> p h r e", h=4, r=128
                        )[:, :, :, 4 * j:4 * j + 4]
                        in_ap = ps.rearrange(
                            "p (e h r) -> p h r e", e=4, h=4)
                        bias_ap = bqkv_sb[:, ti0:ti0 + 8].rearrange(
                            "p (a e b) -> p a b e", a=1, b=2
                        )[:, :, 0:1, :].broadcast_to([128, 4, 128, 4])
                        nc.vector.tensor_tensor(
                            out=out_ap, in0=in_ap, in1=bias_ap,
                            op=mybir.AluOpType.add)
                    if qi == 3:
                        # V section complete: V^T via DMA XBAR transposes
                        for hl in range(HPC):
                            emit_transposes(hl)

            ps_sc_cm = tc.tile_pool(name="ps_sc", bufs=3, space="PSUM")
            ps_po_cm = tc.tile_pool(name="ps_po", bufs=4, space="PSUM")
            ps_se_cm = tc.tile_pool(name="ps_se", bufs=1, space="PSUM")
            ps_sc = ps_sc_cm.__enter__()
            ps_po = ps_po_cm.__enter__()
            ps_se = ps_se_cm.__enter__()

            # normalized attention out, PERMUTED cols (= hl*1024 + cb*128 + r)
            oh = [ohpool.tile([128, 4096], BDT, tag="oh", name=f"oh{c}")
                  for c in range(2)]

            def emit_outproj(hl):
                ys = wpool.tile([128, 256], FDT, tag="ys", name="ys")
                # c-major per ch-half: two accumulation groups in separate
                # PSUM banks so the first half's writeback overlaps the
                # second half's matmuls
                for ch in range(2):
                    yp = ps_po.tile([128, 512], FDT, tag="po", name="yp")
                    for jj in range(16):
                        c, cb = divmod(jj, 8)
                        j = 2 * cb + c
                        mm(
                            yp[:, 0:128],
                            oh[c][:, hl * 1024 + cb * 128:
                                  hl * 1024 + cb * 128 + 128],
                            wot_sb[:, j * 256 + ch * 128:
                                   j * 256 + ch * 128 + 128],
                            start=(jj == 0), stop=(jj == 15),
                        )
                    nc.vector.tensor_copy(out=ys[:, ch * 128:ch * 128 + 128],
                                          in_=yp[:, 0:128])
                    nc.sync.dma_start(
                        out=y[hl * 128:(hl + 1) * 128,
                              ch * 128:ch * 128 + 128],
                        in_=ys[:, ch * 128:ch * 128 + 128],
                    )

            def emit_normalize_half(hl, qj, po, se, rc, h2):
                # query columns [256*h2, 256*h2+256) of this qj window are
                # final once key block 4*qj + 2*h2 + 1 has accumulated
                lo, hi = 256 * h2, 256 * h2 + 256
                nc.vector.reciprocal(out=rc[:, lo:hi], in_=se[:, lo:hi])
                for c in range(2):
                    out_ap = oh[c].rearrange(
                        "p (h cb r) -> p h cb r", h=4, cb=8
                    )[:, hl, :, 64 * qj + 32 * h2:64 * qj + 32 * h2 + 32]
                    nc.vector.tensor_mul(
                        out=out_ap,
                        in0=po[c][:, :].rearrange(
                            "p (th cb) -> p cb th", cb=8)[
                            :, :, 32 * h2:32 * h2 + 32],
                        in1=rc[:, :].rearrange(
                            "p (th cb) -> p cb th", cb=8)[
                            :, :, 32 * h2:32 * h2 + 32],
                    )

            # ---- attention, natural order, causal-suffix matmuls ----
            # Software-pipelined emission: scores for block k+1 are emitted
            # BEFORE PV/se of block k, so the (strictly in-order) PE covers
            # each block's score->exp->PV latency with the next block's
            # score matmuls.
            def emit_scores(hl, qj, k, vkt):
                p_t = max(0, 128 * (k - 4 * qj))
                q_lo = hl * 1024 + 512 * qj
                sp = ps_sc.tile([128, 512], FDT, tag="ps", name="score")
                pt = ptpool.tile([128, 512], BDT, tag="pt", name="pt")
                mm(sp[:, p_t:512],
                   kt[0][:, hl * 1024 + k * 128:hl * 1024 + k * 128 + 128],
                   qt[0][:, q_lo + p_t:q_lo + 512],
                   start=True, stop=False)
                if k >= 4 * qj:
                    mm(sp[:, p_t:p_t + 128], idn_sb, tri_sb,
                       start=False, stop=False)
                mm(sp[:, p_t:512],
                   kt[1][:, hl * 1024 + k * 128:hl * 1024 + k * 128 + 128],
                   qt[1][:, q_lo + p_t:q_lo + 512],
                   start=False, stop=True)
                nc.scalar.activation(pt[:, p_t:512], sp[:, p_t:512],
                                     EXP, scale=1.0 / 16.0)
                return pt

            def emit_pv_se(hl, qj, k, vkt, po, se, pt):
                p_t = max(0, 128 * (k - 4 * qj))
                kmax = 4 * qj + 3
                for c in range(2):
                    mm(
                        po[c][:, p_t:512],
                        vkt[:, c * 1024 + k * 128:c * 1024 + k * 128 + 128],
                        pt[:, p_t:512],
                        start=(k == 0), stop=(k == kmax),
                    )
                mm(
                    se[:, p_t:512], onc_sb, pt[:, p_t:512],
                    start=(k == 0), stop=(k == kmax),
                )

            for hl in range(HPC):
                vkt = vk[hl]
                for qj in range(2):
                    kmax = 4 * qj + 3
                    po = [ps_po.tile([128, 512], FDT, tag="po", name=f"po{c}")
                          for c in range(2)]
                    se = ps_se.tile([128, 512], FDT, tag="se", name="se")
                    rc = wpool.tile([128, 512], FDT, tag="rc", name="rc")
                    pts = {0: emit_scores(hl, qj, 0, vkt),
                           1: emit_scores(hl, qj, 1, vkt)}
                    for k in range(kmax + 1):
                        emit_pv_se(hl, qj, k, vkt, po, se, pts.pop(k))
                        if k + 2 <= kmax:
                            pts[k + 2] = emit_scores(hl, qj, k + 2, vkt)
                        # query half [0,256) of this window is final after
                        # block 4qj+1; normalize it while the rest computes
                        if k == 4 * qj + 1:
                            emit_normalize_half(hl, qj, po, se, rc, 0)
                        # run the previous head's output projection while
                        # DVE normalizes it (one head late in the PE stream)
                        if hl > 0 and qj == 0 and k == 1:
                            emit_outproj(hl - 1)
                    emit_normalize_half(hl, qj, po, se, rc, 1)

            emit_outproj(HPC - 1)
            ps_se_cm.__exit__(None, None, None)
            ps_po_cm.__exit__(None, None, None)
            ps_sc_cm.__exit__(None, None, None)
    nc.finalize()
    return nc


def _prep_inputs(x, w_attn, b_attn, w_out):
    E8 = ml_dtypes.float8_e4m3

    # shared diagonal-block causal mask, natural order: masked iff key > query
    r = np.arange(128)
    tri_arr = np.where(r[:, None] <= r[None, :], 0.0, NEG).astype(
        ml_dtypes.bfloat16)
    bqkv_arr = np.zeros((128, 56), dtype=np.float32)  # [V, Q, K] tile order
    bqkv_arr[:, 0:48] = np.concatenate(
        [b_attn[4096:6144], b_attn[0:2048], b_attn[2048:4096]]
    ).reshape(48, 128).T.astype(np.float32)
    cst_arr = np.ascontiguousarray(np.concatenate(
        [np.eye(128, dtype=ml_dtypes.bfloat16),
         tri_arr,
         np.ones((128, 128), ml_dtypes.bfloat16)], axis=1))

    # weights, sections [V, Q, K] (all unscaled; 1/16 folded into exp scale)
    wall = np.concatenate(
        [w_attn[4096:6144], w_attn[0:2048], w_attn[2048:4096]])  # (6144, 256)
    # w8[p, ti*256 + i*128 + m] = wall[sec*2048 + cb*256 + c*128 + m, i*128+p]
    wt = wall.reshape(3, 8, 2, 128, 2, 128)        # [s, cb, c, m, i, p]
    wfull = np.ascontiguousarray(
        wt.transpose(5, 0, 1, 2, 4, 3).reshape(128, 12288)).astype(np.float32)
    w8_arr = wfull.astype(E8)
    w8_f32 = w8_arr.astype(np.float32)
    wr8_arr = ((wfull - w8_f32) * RS).astype(E8)

    wot_arr = np.ascontiguousarray(
        w_out.T.reshape(16, 128, 256).transpose(1, 0, 2).reshape(128, 4096)
    ).astype(ml_dtypes.bfloat16)

    in_maps = []
    for core in range(NCORES):
        b, g = divmod(core, 2)
        xt = np.ascontiguousarray(x[b, 512 * g:512 * (g + 1)].T)  # (256, 512)
        xp = np.ascontiguousarray(
            np.concatenate([xt[:128], xt[128:]], axis=1)).astype(np.float32)
        x8_arr = xp.astype(E8)
        x8_f32 = x8_arr.astype(np.float32)
        xr8_arr = (xp - x8_f32).astype(E8)       # unscaled: pairs with w8
        x8s_arr = (x8_f32 / RS).astype(E8)       # x8/8: pairs with wr8 (*8)
        in_maps.append({
            "xpk": np.ascontiguousarray(
                np.concatenate([x8_arr, xr8_arr, x8s_arr], axis=1)),
            "w8p": w8_arr,
            "wr8p": wr8_arr,
            "wot": wot_arr,
            "cst": cst_arr,
            "bqkv": bqkv_arr,
        })
    return in_maps


def kernel(x, w_attn, b_attn, w_out, b_out):
    x = np.asarray(x, dtype=np.float32)
    w_attn = np.asarray(w_attn, dtype=np.float32)
    b_attn = np.asarray(b_attn, dtype=np.float32)
    w_out = np.asarray(w_out, dtype=np.float32)
    b_out = np.asarray(b_out, dtype=np.float32)

    if "nc" not in _cache:
        _cache["nc"] = _build()
    nc = _cache["nc"]

    in_maps = _prep_inputs(x, w_attn, b_attn, w_out)
    res = run_bass_kernel_spmd(nc, in_maps, list(range(NCORES))).results

    out = np.empty((BS, SQL, EDIM), dtype=np.float32)
    for core in range(NCORES):
        b, g = divmod(core, 2)
        out[b, 512 * g:512 * (g + 1)] = res[core]["y"]
    out += b_out
    return out


# revision 19
# speedup vs baseline: 1.1447x; 1.1447x over previous
"""Masked multi-head attention on 8 NeuronCores (faithful torch raw-view semantics).

The reference reshapes (bs, sql, nh*edim) -> (bs, nh, sql, edim) as a RAW VIEW:
head h's length-1024 pseudo-sequence comes from x rows 128h..128h+127, each row
contributing 8 pseudo-positions s' = 8r + cb (cb = 256-col block of the
projection). Work splits into 32 independent (batch, head) pairs -> 4 per core.

v3 design (vs the fp32r/bf16 baseline):
- Projections run as fp8e4m3 DoubleRow matmuls with a 3-product residual
  scheme (x8*w8 + rx8*(w8/8) + (x8/8)*rw8, residuals pre-scaled by 8 on the
  host): 256-deep contraction in one PE pass at 0.5 cycles/row, 1.5N rows
  per tile vs 2N for bf16. All six operand tensors are host-prepped fp8 -
  zero on-device conversion cost.
- Q is NOT pre-scaled; the 1/16 softmax scale is folded into the exp
  activation's scale argument (fp8 weights would underflow at sigma=1/256).
- Scores/PV/denominator in bf16 (removes the fp32r >=256-moving-column
  padding). Causal triangle is ADDED ON THE PE (idn x tri accumulate into
  the score PSUM) instead of DVE tensor_adds.
- V^T per-head key-block transposes run on the DMA XBAR (dma_start_transpose,
  14ns/16x128-tile) instead of PE transposes + DVE copybacks.
- PSUM: score tiles [128,512] bufs=2, po bufs=4 (so consecutive qj rounds
  overlap), se, yp.
- Output projection per head (full r), emitted one head late into the PE
  stream so it runs while DVE normalizes the next head's first rounds.
"""

import sys

sys.path.insert(0, "/opt/trn_rl_repo")

import ml_dtypes
import numpy as np

from concourse import bacc, mybir
from concourse.tile import TileContext
from concourse.bass_utils import run_bass_kernel_spmd

EDIM = 256
BS = 4
SQL = 1024
HPC = 4           # heads per core
NCORES = 8
FDT = mybir.dt.float32
BDT = mybir.dt.bfloat16
QDT = mybir.dt.float8e4
NEG = -1.0e30
RS = 8.0          # fp8 residual pre-scale
DR = mybir.MatmulPerfMode.DoubleRow

_cache = {}


def _build():
    nc = bacc.Bacc(dynamic_dma_scratch_size=512)

    # fp8 moving operands (x^T), layout [p, i*512+n] = x_slice.T[i*128+p, n]
    # packed [x8 | fp8(rx) | x8/8]
    xpk = nc.declare_dram_parameter("xpk", [128, 3072], QDT, isOutput=False)
    # fp8 stationary weights, col = ti*256 + i*128 + m; ti = s*16 + cb*2 + c
    # sections s in [V, Q, K]; wr8 = (w - w8)*8
    w8p = nc.declare_dram_parameter("w8p", [128, 12288], QDT, isOutput=False)
    wr8p = nc.declare_dram_parameter("wr8p", [128, 12288], QDT, isOutput=False)
    # bf16 constants: idn | tri | ones
    cst = nc.declare_dram_parameter("cst", [128, 384], BDT, isOutput=False)
    bqkv = nc.declare_dram_parameter("bqkv", [128, 48], FDT, isOutput=False)
    wot = nc.declare_dram_parameter("wot", [128, 4096], BDT, isOutput=False)
    y = nc.declare_dram_parameter("y", [512, 256], FDT, isOutput=True)

    EXP = mybir.ActivationFunctionType.Exp

    with TileContext(nc) as tc:
        with (
            tc.tile_pool(name="const", bufs=1) as cpool,
            tc.tile_pool(name="qk", bufs=1) as qkpool,
            tc.tile_pool(name="ohp", bufs=2) as ohpool,
            tc.tile_pool(name="vkp", bufs=4) as vkpool,
            tc.tile_pool(name="ptp", bufs=4) as ptpool,
            tc.tile_pool(name="work", bufs=2) as wpool,
            tc.tile_pool(name="ps_sc", bufs=3, space="PSUM") as ps_sc,
            tc.tile_pool(name="ps_po", bufs=4, space="PSUM") as ps_po,
            tc.tile_pool(name="ps_se", bufs=1, space="PSUM") as ps_se,
        ):
            def mm(out, lhsT, rhs, **kw):
                nc.tensor.matmul(out, lhsT, rhs, **kw)

            def tile_of(pool, name, shape, dt=FDT, tag=None):
                return pool.tile(shape, dt, tag=tag or name, name=name)

            # ---- input DMA: split across the two HWDGE queues (SP, Act) in
            # need order; W tensors per-section so arrival chases the PE.
            xpk_sb = tile_of(cpool, "xpk", [128, 3072], dt=QDT)
            w8_sb = tile_of(cpool, "w8", [128, 12288], dt=QDT)
            wr8_sb = tile_of(cpool, "wr8", [128, 12288], dt=QDT)
            cst_sb = tile_of(cpool, "cst", [128, 384], dt=BDT)
            bqkv_sb = tile_of(cpool, "bqkv", [128, 48])
            wot_sb = tile_of(cpool, "wot", [128, 4096], dt=BDT)

            # SP first (the Act queue's SEQ is blocked ~1.3us by the act
            # table load): x pack + first V chunks, then Q; Act: rest of V,
            # K, consts. Transfers serialize on the single DMA_ENGINES
            # device in roughly this interleaved order.
            nc.sync.dma_start(out=xpk_sb[:, :], in_=xpk[:, :])
            nc.scalar.dma_start(out=bqkv_sb[:, :], in_=bqkv[:, :])
            nc.scalar.dma_start(out=cst_sb[:, :], in_=cst[:, :])
            for lo in range(0, 12288, 2048):
                nc.sync.dma_start(out=w8_sb[:, lo:lo + 2048],
                                  in_=w8p[:, lo:lo + 2048])
                nc.sync.dma_start(out=wr8_sb[:, lo:lo + 2048],
                                  in_=wr8p[:, lo:lo + 2048])
            nc.sync.dma_start(out=wot_sb[:, :], in_=wot[:, :])

            idn_sb = cst_sb[:, 0:128]
            tri_sb = cst_sb[:, 128:256]
            onc_sb = cst_sb[:, 256:384]

            # fp8 moving views [p, i, n] for DoubleRow
            x8_r = xpk_sb[:, 0:1024].rearrange("p (i n) -> p i n", i=2)
            xr8_r = xpk_sb[:, 1024:2048].rearrange("p (i n) -> p i n", i=2)
            x8s_r = xpk_sb[:, 2048:3072].rearrange("p (i n) -> p i n", i=2)

            # PE warmup: throwaway matmuls on a DVE-memset tile ramp the PE
            # to full pstate while the first weight DMAs are in flight.
            wup = wpool.tile([32, 640], BDT, tag="wup", name="wup", bufs=1)
            nc.gpsimd.memset(wup[:, 0:320], 0.0)
            nc.gpsimd.memset(wup[:, 320:640], 0.0)
            wps = ps_sc.tile([128, 512], FDT, tag="ps", name="warm")
            for _ in range(8):
                mm(wps[:, :], wup[:, 512:640], wup[:, 0:512])

            # natural-order projections: col = h*1024 + 8r + cb
            qt = [qkpool.tile([128, 4096], BDT, tag=f"qt{c}", name=f"qt{c}")
                  for c in range(2)]
            kt = [qkpool.tile([128, 4096], BDT, tag=f"kt{c}", name=f"kt{c}")
                  for c in range(2)]
            vt = [qkpool.tile([128, 4096], BDT, tag=f"vt{c}", name=f"vt{c}")
                  for c in range(2)]

            # ---- P1: V, Q, K projections as fp8 DoubleRow units ----
            units = [(s, cb, c) for s in range(3) for cb in range(8)
                     for c in range(2)]
            vk = {}

            def emit_transposes(hl):
                vkt = vkpool.tile([128, 2048], BDT, tag="vk", name=f"vk{hl}")
                for c in range(2):
                    nc.sync.dma_start_transpose(
                        out=vkt[:, c * 1024:(c + 1) * 1024].rearrange(
                            "p (b d) -> p b d", b=8),
                        in_=vt[c][:, hl * 1024:(hl + 1) * 1024],
                    )
                vk[hl] = vkt

            for ui, (s, cb, c) in enumerate(units):
                ti = s * 16 + cb * 2 + c
                pool = ps_sc if ui % 2 == 0 else ps_po
                ps = pool.tile([128, 512], FDT,
                               tag="ps" if ui % 2 == 0 else "po",
                               name="proj")
                wof = ti * 256
                w_a = w8_sb[:, wof:wof + 256].rearrange("p (i m) -> p i m",
                                                        i=2)
                w_c = wr8_sb[:, wof:wof + 256].rearrange("p (i m) -> p i m",
                                                         i=2)
                mm(ps[:, :], w_a, x8_r, start=True, stop=False, perf_mode=DR)
                mm(ps[:, :], w_a, xr8_r, start=False, stop=False,
                   perf_mode=DR)
                mm(ps[:, :], w_c, x8s_r, start=False, stop=True, perf_mode=DR)
                dst = (vt, qt, kt)[s]
                out_ap = dst[c].rearrange(
                    "p (h r e) -> p h r e", h=4, r=128)[:, :, :, cb]
                in_ap = ps.rearrange("p (h r) -> p h r", h=4)
                if ti % 2 == 0:
                    nc.scalar.add(out_ap, in_ap, add=bqkv_sb[:, ti:ti + 1])
                else:
                    nc.vector.tensor_scalar_add(
                        out=out_ap, in0=in_ap,
                        scalar1=bqkv_sb[:, ti:ti + 1],
                    )
                if ui == 15:
                    # V section complete: V^T via DMA XBAR transposes
                    for hl in range(HPC):
                        emit_transposes(hl)

            # normalized attention out, PERMUTED cols (= hl*1024 + cb*128 + r)
            oh = [ohpool.tile([128, 4096], BDT, tag="oh", name=f"oh{c}")
                  for c in range(2)]

            def emit_outproj(hl):
                ys = wpool.tile([128, 256], FDT, tag="ys", name="ys")
                # c-major per ch-half: two accumulation groups in separate
                # PSUM banks so the first half's writeback overlaps the
                # second half's matmuls
                for ch in range(2):
                    yp = ps_po.tile([128, 512], FDT, tag="po", name="yp")
                    for jj in range(16):
                        c, cb = divmod(jj, 8)
                        j = 2 * cb + c
                        mm(
                            yp[:, 0:128],
                            oh[c][:, hl * 1024 + cb * 128:
                                  hl * 1024 + cb * 128 + 128],
                            wot_sb[:, j * 256 + ch * 128:
                                   j * 256 + ch * 128 + 128],
                            start=(jj == 0), stop=(jj == 15),
                        )
                    nc.vector.tensor_copy(out=ys[:, ch * 128:ch * 128 + 128],
                                          in_=yp[:, 0:128])
                    nc.sync.dma_start(
                        out=y[hl * 128:(hl + 1) * 128,
                              ch * 128:ch * 128 + 128],
                        in_=ys[:, ch * 128:ch * 128 + 128],
                    )

            def emit_normalize_half(hl, qj, po, se, rc, h2):
                # query columns [256*h2, 256*h2+256) of this qj window are
                # final once key block 4*qj + 2*h2 + 1 has accumulated
                lo, hi = 256 * h2, 256 * h2 + 256
                nc.vector.reciprocal(out=rc[:, lo:hi], in_=se[:, lo:hi])
                for c in range(2):
                    out_ap = oh[c].rearrange(
                        "p (h cb r) -> p h cb r", h=4, cb=8
                    )[:, hl, :, 64 * qj + 32 * h2:64 * qj + 32 * h2 + 32]
                    nc.vector.tensor_mul(
                        out=out_ap,
                        in0=po[c][:, :].rearrange(
                            "p (th cb) -> p cb th", cb=8)[
                            :, :, 32 * h2:32 * h2 + 32],
                        in1=rc[:, :].rearrange(
                            "p (th cb) -> p cb th", cb=8)[
                            :, :, 32 * h2:32 * h2 + 32],
                    )

            # ---- attention, natural order, causal-suffix matmuls ----
            # Software-pipelined emission: scores for block k+1 are emitted
            # BEFORE PV/se of block k, so the (strictly in-order) PE covers
            # each block's score->exp->PV latency with the next block's
            # score matmuls.
            def emit_scores(hl, qj, k, vkt):
                p_t = max(0, 128 * (k - 4 * qj))
                q_lo = hl * 1024 + 512 * qj
                sp = ps_sc.tile([128, 512], FDT, tag="ps", name="score")
                pt = ptpool.tile([128, 512], BDT, tag="pt", name="pt")
                mm(sp[:, p_t:512],
                   kt[0][:, hl * 1024 + k * 128:hl * 1024 + k * 128 + 128],
                   qt[0][:, q_lo + p_t:q_lo + 512],
                   start=True, stop=False)
                if k >= 4 * qj:
                    mm(sp[:, p_t:p_t + 128], idn_sb, tri_sb,
                       start=False, stop=False)
                mm(sp[:, p_t:512],
                   kt[1][:, hl * 1024 + k * 128:hl * 1024 + k * 128 + 128],
                   qt[1][:, q_lo + p_t:q_lo + 512],
                   start=False, stop=True)
                nc.scalar.activation(pt[:, p_t:512], sp[:, p_t:512],
                                     EXP, scale=1.0 / 16.0)
                return pt

            def emit_pv_se(hl, qj, k, vkt, po, se, pt):
                p_t = max(0, 128 * (k - 4 * qj))
                kmax = 4 * qj + 3
                for c in range(2):
                    mm(
                        po[c][:, p_t:512],
                        vkt[:, c * 1024 + k * 128:c * 1024 + k * 128 + 128],
                        pt[:, p_t:512],
                        start=(k == 0), stop=(k == kmax),
                    )
                mm(
                    se[:, p_t:512], onc_sb, pt[:, p_t:512],
                    start=(k == 0), stop=(k == kmax),
                )

            for hl in range(HPC):
                vkt = vk[hl]
                for qj in range(2):
                    kmax = 4 * qj + 3
                    po = [ps_po.tile([128, 512], FDT, tag="po", name=f"po{c}")
                          for c in range(2)]
                    se = ps_se.tile([128, 512], FDT, tag="se", name="se")
                    rc = wpool.tile([128, 512], FDT, tag="rc", name="rc")
                    pts = {0: emit_scores(hl, qj, 0, vkt),
                           1: emit_scores(hl, qj, 1, vkt)}
                    for k in range(kmax + 1):
                        emit_pv_se(hl, qj, k, vkt, po, se, pts.pop(k))
                        if k + 2 <= kmax:
                            pts[k + 2] = emit_scores(hl, qj, k + 2, vkt)
                        # query half [0,256) of this window is final after
                        # block 4qj+1; normalize it while the rest computes
                        if k == 4 * qj + 1:
                            emit_normalize_half(hl, qj, po, se, rc, 0)
                        # run the previous head's output projection while
                        # DVE normalizes it (one head late in the PE stream)
                        if hl > 0 and qj == 0 and k == 1:
                            emit_outproj(hl - 1)
                    emit_normalize_half(hl, qj, po, se, rc, 1)

            emit_outproj(HPC - 1)
    nc.finalize()
    return nc


def _prep_inputs(x, w_attn, b_attn, w_out):
    E8 = ml_dtypes.float8_e4m3

    # shared diagonal-block causal mask, natural order: masked iff key > query
    r = np.arange(128)
    tri_arr = np.where(r[:, None] <= r[None, :], 0.0, NEG).astype(
        ml_dtypes.bfloat16)
    bqkv_arr = np.ascontiguousarray(np.concatenate(
        [b_attn[4096:6144], b_attn[0:2048], b_attn[2048:4096]]
    ).reshape(48, 128).T).astype(np.float32)  # [V, Q, K] tile order
    cst_arr = np.ascontiguousarray(np.concatenate(
        [np.eye(128, dtype=ml_dtypes.bfloat16),
         tri_arr,
         np.ones((128, 128), ml_dtypes.bfloat16)], axis=1))

    # weights, sections [V, Q, K] (all unscaled; 1/16 folded into exp scale)
    wall = np.concatenate(
        [w_attn[4096:6144], w_attn[0:2048], w_attn[2048:4096]])  # (6144, 256)
    # w8[p, ti*256 + i*128 + m] = wall[sec*2048 + cb*256 + c*128 + m, i*128+p]
    wt = wall.reshape(3, 8, 2, 128, 2, 128)        # [s, cb, c, m, i, p]
    wfull = np.ascontiguousarray(
        wt.transpose(5, 0, 1, 2, 4, 3).reshape(128, 12288)).astype(np.float32)
    w8_arr = wfull.astype(E8)
    w8_f32 = w8_arr.astype(np.float32)
    wr8_arr = ((wfull - w8_f32) * RS).astype(E8)

    wot_arr = np.ascontiguousarray(
        w_out.T.reshape(16, 128, 256).transpose(1, 0, 2).reshape(128, 4096)
    ).astype(ml_dtypes.bfloat16)

    in_maps = []
    for core in range(NCORES):
        b, g = divmod(core, 2)
        xt = np.ascontiguousarray(x[b, 512 * g:512 * (g + 1)].T)  # (256, 512)
        xp = np.ascontiguousarray(
            np.concatenate([xt[:128], xt[128:]], axis=1)).astype(np.float32)
        x8_arr = xp.astype(E8)
        x8_f32 = x8_arr.astype(np.float32)
        xr8_arr = (xp - x8_f32).astype(E8)       # unscaled: pairs with w8
        x8s_arr = (x8_f32 / RS).astype(E8)       # x8/8: pairs with wr8 (*8)
        in_maps.append({
            "xpk": np.ascontiguousarray(
                np.concatenate([x8_arr, xr8_arr, x8s_arr], axis=1)),
            "w8p": w8_arr,
            "wr8p": wr8_arr,
            "wot": wot_arr,
            "cst": cst_arr,
            "bqkv": bqkv_arr,
        })
    return in_maps


def kernel(x, w_attn, b_attn, w_out, b_out):
    x = np.asarray(x, dtype=np.float32)
    w_attn = np.asarray(w_attn, dtype=np.float32)
    b_attn = np.asarray(b_attn, dtype=np.float32)
    w_out = np.asarray(w_out, dtype=np.float32)
    b_out = np.asarray(b_out, dtype=np.float32)

    if "nc" not in _cache:
        _cache["nc"] = _build()
    nc = _cache["nc"]

    in_maps = _prep_inputs(x, w_attn, b_attn, w_out)
    res = run_bass_kernel_spmd(nc, in_maps, list(range(NCORES))).results

    out = np.empty((BS, SQL, EDIM), dtype=np.float32)
    for core in range(NCORES):
        b, g = divmod(core, 2)
        out[b, 512 * g:512 * (g + 1)] = res[core]["y"]
    out += b_out
    return out
